# revision 3
# baseline (speedup 1.0000x reference)
"""EGNN (2-layer) on 8 Trainium2 NeuronCores.

Device (Bass/Tile, SPMD x8): the dense per-edge MLP chain
  msg_h = silu(lin(silu(lin(f, ew1, eb1)), ew2, eb2));  coord_s = lin(silu(lin(msg_h, cw1, cb1)), cw2)
over E/8 edges per core, feature-major layout ([feat, edge] tiles, N=512 matmuls,
2048-wide activation ops). Host (numpy): edge gathers, segment sums, node MLP,
gelu+layernorm, output head.
"""
import os
import sys

import numpy as np

for _p in ("/opt/trn_rl_repo", "/root/.axon_site/_ro/trn_rl_repo"):
    if os.path.isdir(_p) and _p not in sys.path:
        sys.path.insert(0, _p)

import jax

jax.config.update("jax_compilation_cache_dir", "/tmp/jax_neff_cache")
jax.config.update("jax_persistent_cache_min_entry_size_bytes", -1)
jax.config.update("jax_persistent_cache_min_compile_time_secs", 0.0)

N = 100000
E = 1600000
C = 64
NOUT = 32
L = 2
NCORES = 8
TILE = 2048
MM = 512  # max fp32 moving free dim / psum bank
EP = 200704  # per-core padded edge count (= 98 * 2048)
NCHUNK = EP // TILE

_COMPILED = None


def _silu(x):
    return x / (1.0 + np.exp(-x))


def _gelu(x):
    from scipy.special import erf

    return (0.5 * x * (1.0 + erf(x / np.sqrt(2.0)))).astype(np.float32)


def _build_bass():
    import concourse.bacc as bacc
    import concourse.bass as bass
    import concourse.mybir as mybir
    from concourse import tile

    f32 = mybir.dt.float32
    nc = bacc.Bacc(None, target_bir_lowering=False, debug=False)

    ft_hi = nc.declare_dram_parameter("ft_hi", [128, EP], f32, isOutput=False)
    ft_lo = nc.declare_dram_parameter("ft_lo", [2, EP], f32, isOutput=False)
    w1a = nc.declare_dram_parameter("w1a", [128, C], f32, isOutput=False)
    w1b = nc.declare_dram_parameter("w1b", [2, C], f32, isOutput=False)
    b1 = nc.declare_dram_parameter("b1", [C, 1], f32, isOutput=False)
    w2 = nc.declare_dram_parameter("w2", [C, C], f32, isOutput=False)
    b2 = nc.declare_dram_parameter("b2", [C, 1], f32, isOutput=False)
    cw1 = nc.declare_dram_parameter("cw1", [C, C], f32, isOutput=False)
    cb1 = nc.declare_dram_parameter("cb1", [C, 1], f32, isOutput=False)
    cw2 = nc.declare_dram_parameter("cw2", [C, 1], f32, isOutput=False)
    msg = nc.declare_dram_parameter("msg", [C, EP], f32, isOutput=True)
    cs = nc.declare_dram_parameter("cs", [1, EP], f32, isOutput=True)

    Silu = mybir.ActivationFunctionType.Silu

    with tile.TileContext(nc) as tc:
        with (
            tc.tile_pool(name="wpool", bufs=1) as wp,
            tc.tile_pool(name="io", bufs=3) as io,
            tc.tile_pool(name="act", bufs=3) as ap,
            tc.tile_pool(name="psum", bufs=2, space=bass.MemorySpace.PSUM) as pp,
        ):
            tw1a = wp.tile([128, C], f32, tag="w1a")
            tw1b = wp.tile([2, C], f32, tag="w1b")
            tb1 = wp.tile([C, 1], f32, tag="b1")
            tw2 = wp.tile([C, C], f32, tag="w2")
            tb2 = wp.tile([C, 1], f32, tag="b2")
            tcw1 = wp.tile([C, C], f32, tag="cw1")
            tcb1 = wp.tile([C, 1], f32, tag="cb1")
            tcw2 = wp.tile([C, 1], f32, tag="cw2")
            for t, d in (
                (tw1a, w1a), (tw1b, w1b), (tb1, b1), (tw2, w2), (tb2, b2),
                (tcw1, cw1), (tcb1, cb1), (tcw2, cw2),
            ):
                nc.sync.dma_start(t[:], d[:])

            for j in range(NCHUNK):
                sl = slice(j * TILE, (j + 1) * TILE)
                thi = io.tile([128, TILE], f32, tag="thi")
                tlo = io.tile([2, TILE], f32, tag="tlo")
                nc.sync.dma_start(thi[:], ft_hi[:, sl])
                nc.sync.dma_start(tlo[:], ft_lo[:, sl])

                p1 = pp.tile([C, TILE], f32, tag="pb")
                for k in range(TILE // MM):
                    ms = slice(k * MM, (k + 1) * MM)
                    nc.tensor.matmul(p1[:, ms], tw1a[:], thi[:, ms], start=True, stop=False)
                    nc.tensor.matmul(p1[:, ms], tw1b[:], tlo[:, ms], start=False, stop=True)
                s1 = ap.tile([C, TILE], f32, tag="s1")
                nc.scalar.activation(s1[:], p1[:], Silu, bias=tb1[:])

                p2 = pp.tile([C, TILE], f32, tag="pb")
                for k in range(TILE // MM):
                    ms = slice(k * MM, (k + 1) * MM)
                    nc.tensor.matmul(p2[:, ms], tw2[:], s1[:, ms], start=True, stop=True)
                s2 = ap.tile([C, TILE], f32, tag="s2")
                nc.scalar.activation(s2[:], p2[:], Silu, bias=tb2[:])
                nc.sync.dma_start(msg[:, sl], s2[:])

                p3 = pp.tile([C, TILE], f32, tag="pb")
                for k in range(TILE // MM):
                    ms = slice(k * MM, (k + 1) * MM)
                    nc.tensor.matmul(p3[:, ms], tcw1[:], s2[:, ms], start=True, stop=True)
                s3 = ap.tile([C, TILE], f32, tag="s3")
                nc.scalar.activation(s3[:], p3[:], Silu, bias=tcb1[:])

                p4 = pp.tile([1, TILE], f32, tag="pb")
                for k in range(TILE // MM):
                    ms = slice(k * MM, (k + 1) * MM)
                    nc.tensor.matmul(p4[:, ms], tcw2[:], s3[:, ms], start=True, stop=True)
                s4 = ap.tile([1, TILE], f32, tag="s4")
                nc.vector.tensor_copy(s4[:], p4[:])
                nc.sync.dma_start(cs[:, sl], s4[:])

    nc.finalize()
    return nc


def _get_compiled():
    global _COMPILED
    if _COMPILED is None:
        _COMPILED = _build_bass()
    return _COMPILED


def _edge_mlp_device(f128, flo, weights):
    """f128: [128, Epad] f32, flo: [2, Epad]. Returns msg_h [Epad, C], coord_s [Epad]."""
    from concourse.bass_utils import run_bass_kernel_spmd

    nc = _get_compiled()
    Epad = f128.shape[1]
    assert Epad == EP * NCORES
    w1a, w1b, b1, w2, b2, cw1, cb1, cw2 = weights
    in_maps = []
    for c in range(NCORES):
        sl = slice(c * EP, (c + 1) * EP)
        in_maps.append({
            "ft_hi": np.ascontiguousarray(f128[:, sl]),
            "ft_lo": np.ascontiguousarray(flo[:, sl]),
            "w1a": w1a, "w1b": w1b, "b1": b1, "w2": w2, "b2": b2,
            "cw1": cw1, "cb1": cb1, "cw2": cw2,
        })
    res = run_bass_kernel_spmd(nc, in_maps, core_ids=list(range(NCORES)))
    outs = res.results
    msg = np.concatenate([np.asarray(outs[c]["msg"]) for c in range(NCORES)], axis=1)
    cs = np.concatenate([np.asarray(outs[c]["cs"]) for c in range(NCORES)], axis=1)
    return msg.T.astype(np.float32), cs[0].astype(np.float32)


def _edge_mlp_host(f128, flo, weights):
    w1a, w1b, b1, w2, b2, cw1, cb1, cw2 = weights
    f1 = (f128.T @ w1a) + (flo.T @ w1b) + b1[:, 0]
    s1 = _silu(f1)
    s2 = _silu(s1 @ w2 + b2[:, 0])
    s3 = _silu(s2 @ cw1 + cb1[:, 0])
    cs = s3 @ cw2
    return s2.astype(np.float32), cs[:, 0].astype(np.float32)


def kernel(node_feat, xyz, src, dst, edge_w1, edge_b1, edge_w2, edge_b2,
           coord_w1, coord_b1, coord_w2, node_w1, node_b1, node_w2, node_b2,
           ln_g, ln_b, out_w, out_b):
    node_feat = np.asarray(node_feat, np.float32)
    xyz = np.asarray(xyz, np.float32)
    src = np.asarray(src, np.int32)
    dst = np.asarray(dst, np.int32)

    # sort edges by dst once; segment sums via reduceat
    perm = np.argsort(dst, kind="stable")
    src_s = src[perm]
    dst_s = dst[perm]
    deg = np.bincount(dst_s, minlength=N).astype(np.float32)[:, None]
    starts = np.searchsorted(dst_s, np.arange(N)).astype(np.int64)
    starts_c = np.minimum(starts, E - 1)
    empty = (deg[:, 0] == 0)

    Epad = EP * NCORES
    h = node_feat
    x = xyz

    use_device = os.environ.get("EGNN_HOST_ONLY", "0") != "1"

    for l in range(L):
        x_diff = x[src_s] - x[dst_s]
        radial = np.sum(x_diff * x_diff, axis=1, keepdims=True)
        x_diff = x_diff / (np.sqrt(radial) + 1e-30)

        f128 = np.empty((128, Epad), np.float32)
        f128[:C, :E] = h[src_s].T
        f128[C:, :E] = h[dst_s].T
        f128[:, E:] = 0.0
        flo = np.zeros((2, Epad), np.float32)
        flo[0, :E] = radial[:, 0]

        w = (
            np.ascontiguousarray(edge_w1[l][:128], np.float32),
            np.ascontiguousarray(
                np.concatenate([edge_w1[l][128:129], np.zeros((1, C), np.float32)], 0)),
            np.ascontiguousarray(edge_b1[l][:, None], np.float32),
            np.ascontiguousarray(edge_w2[l], np.float32),
            np.ascontiguousarray(edge_b2[l][:, None], np.float32),
            np.ascontiguousarray(coord_w1[l], np.float32),
            np.ascontiguousarray(coord_b1[l][:, None], np.float32),
            np.ascontiguousarray(coord_w2[l], np.float32),
        )
        if use_device:
            try:
                msg_h, coord_s = _edge_mlp_device(f128, flo, w)
            except Exception as e:  # pragma: no cover - device fallback
                print(f"[kernel] device path failed ({type(e).__name__}: {e}); "
                      f"falling back to host", file=sys.stderr)
                use_device = False
                msg_h, coord_s = _edge_mlp_host(f128, flo, w)
        else:
            msg_h, coord_s = _edge_mlp_host(f128, flo, w)
        msg_h = msg_h[:E]
        coord_s = coord_s[:E]
        msg_x = coord_s[:, None] * x_diff

        h_neigh = np.add.reduceat(msg_h, starts_c, axis=0)
        h_neigh[empty] = 0.0
        x_sum = np.add.reduceat(msg_x, starts_c, axis=0)
        x_sum[empty] = 0.0
        x = x + x_sum / np.maximum(deg, 1.0)

        hcat = np.concatenate([h, h_neigh], axis=1)
        h = _silu(hcat @ node_w1[l] + node_b1[l]) @ node_w2[l] + node_b2[l]
        h = _gelu(h)
        mu = h.mean(axis=1, keepdims=True)
        var = np.mean((h - mu) ** 2, axis=1, keepdims=True)
        h = (h - mu) / np.sqrt(var + 1e-5) * ln_g + ln_b
        h = h.astype(np.float32)

    return (h @ out_w + out_b).astype(np.float32)

